# revision 9
# baseline (speedup 1.0000x reference)
"""Trainium2 Bass kernel for nn_Attention_7945689497706.

Distribution: data-parallel over batch, 2 batch elements per core, weights
replicated, no collectives.

Per-core layout (v3):
  - PE warm-up matmuls during the initial x DMA (HAM un-throttle).
  - x staged in f32 chunks then cast to bf16 once; norm squares/muls run off
    the bf16 copy (xn mul on GPSIMD to unload the DVE).
  - Both batches' sqrt-based norms run before any exp, so the ACT table set
    switches twice total instead of per batch.
  - sims for a head pair run as two concurrent 64-row PE tiles into one
    [128, 2, 1024] psum tile (high_priority keeps the quad adjacent in the
    engine queue); one [128, 2048] exp per (pair, jc) evacuates both heads.
  - p chunks live in a ring of per-jc [128, 2, 1024] bf16 tiles; av replays
    them after the pair's exps (h2-major phases, one [128, 2, 512] av psum).
  - denominator rides the 65th stationary column; bc broadcast + fast
    reciprocal + mul as in the baseline.
  - batch-1 projections interleave into batch-0's attention (and batch-0
    out-proj into batch-1's) to fill the ACT-bound exp bubbles.
"""

import numpy as np

import concourse.bass as bass
import concourse.mybir as mybir
import concourse.tile as tile
from concourse import bacc
from concourse.bass_utils import run_bass_kernel_spmd

F32 = mybir.dt.float32
F32R = mybir.dt.float32r
BF16 = mybir.dt.bfloat16
FP8 = mybir.dt.float8e4
AF = mybir.ActivationFunctionType

NCORES = 8
B = 16
C = 512
N = 1024          # pixels = 32*32
HEADS = 8
DH = 64
NMEM = 4
PB = B // NCORES  # batch elements per core
CT = C // 128     # channel partition-tiles
VW = HEADS * (DH + 1)  # per head [v | ones] = 65


def _build():
    nc = bacc.Bacc()
    x_ext = nc.declare_dram_parameter("x", [PB, C, N], F32, isOutput=False)
    wqkvt_ext = nc.declare_dram_parameter("wqkvt", [C, 3 * C], F32, isOutput=False)
    wot_ext = nc.declare_dram_parameter("wot", [C, C], F32, isOutput=False)
    gammat_ext = nc.declare_dram_parameter("gammat", [128, CT], F32, isOutput=False)
    memk_ext = nc.declare_dram_parameter("memk", [128, HEADS, NMEM], F32, isOutput=False)
    memv_ext = nc.declare_dram_parameter("memv", [128, 2, VW], F32, isOutput=False)
    out_ext = nc.declare_dram_parameter("out", [PB, C, N], F32, isOutput=True)

    with tile.TileContext(nc) as tc:
        with (
            tc.tile_pool(name="const", bufs=1) as const,
            tc.tile_pool(name="stage", bufs=2) as stage,
            tc.tile_pool(name="xbp", bufs=1) as xbp,
            tc.tile_pool(name="data", bufs=1) as data,
            tc.tile_pool(name="atp", bufs=2) as atp,
            tc.tile_pool(name="qp", bufs=2) as qp,
            tc.tile_pool(name="pp", bufs=12) as pp,
            tc.tile_pool(name="pm", bufs=4) as pm,
            tc.tile_pool(name="avs", bufs=2) as avsp,
            tc.tile_pool(name="rp", bufs=1) as rp,
            tc.tile_pool(name="ob", bufs=1) as obp,
            tc.tile_pool(name="qkv_ps", bufs=2, space="PSUM") as qkv_ps,
            tc.tile_pool(name="sim_ps", bufs=1, space="PSUM") as sim_ps,
            tc.tile_pool(name="av_ps", bufs=1, space="PSUM") as av_ps,
        ):
            # ---------------- constants + PE warm-up ----------------
            ones128 = const.tile([128, 128], BF16, tag="ones128")
            nc.vector.memset(ones128, 1.0)
            for w in range(60):
                wp = qkv_ps.tile([128, 512], F32, tag="q")
                nc.tensor.matmul(wp[:, 0:128], ones128, ones128, start=True, stop=True)

            wqkv = const.tile([128, CT, 3 * C], BF16, tag="wqkv")
            wo = const.tile([128, CT, C], BF16, tag="wo")
            g1 = const.tile([128, CT], F32, tag="g1")
            g1q = const.tile([128, CT], F32, tag="g1q")
            ones1 = const.tile([128, 64], F32R, tag="ones1")
            kTp = const.tile([128, HEADS, 1028], BF16, tag="kTp")
            vextA = const.tile([128, 8, VW], BF16, tag="vextA")
            vextB = const.tile([128, 8, VW], BF16, tag="vextB")
            vmem = const.tile([128, 2, VW], FP8, tag="vmem")
            vexts = [vextA, vextB]

            gsb = const.tile([128, CT], F32, tag="gsb")
            nc.sync.dma_start(out=gsb, in_=gammat_ext[:, :])
            nc.scalar.activation(out=g1, in_=gsb, func=AF.Copy, bias=1.0)
            nc.scalar.activation(out=g1q, in_=gsb, func=AF.Copy, bias=1.0)
            nc.scalar.mul(out=g1q, in_=g1q, mul=DH ** -0.5)
            nc.vector.memset(ones1.bitcast(F32), 1.0)

            def weight_prep():
                nc.gpsimd.memset(kTp, 0.0)
                for t in range(CT):
                    for half in range(2):
                        ws = stage.tile([128, 1024], F32, tag="ws")
                        c0 = half * 768
                        nc.sync.dma_start(out=ws[:, 0:768],
                                          in_=wqkvt_ext[t * 128:(t + 1) * 128, c0:c0 + 768])
                        if half == 0:
                            nc.gpsimd.tensor_scalar_mul(
                                out=wqkv[:, t, 0:C], in0=ws[:, 0:C], scalar1=g1q[:, t:t + 1])
                            nc.gpsimd.tensor_scalar_mul(
                                out=wqkv[:, t, C:768], in0=ws[:, C:768], scalar1=g1[:, t:t + 1])
                        else:
                            nc.gpsimd.tensor_scalar_mul(
                                out=wqkv[:, t, 768:3 * C], in0=ws[:, 0:768],
                                scalar1=g1[:, t:t + 1])
                for t in range(CT):
                    ws = stage.tile([128, 1024], F32, tag="ws")
                    nc.sync.dma_start(out=ws[:, 0:C], in_=wot_ext[t * 128:(t + 1) * 128, :])
                    nc.vector.tensor_copy(out=wo[:, t, :], in_=ws[:, 0:C])
                ws = stage.tile([128, 1024], F32, tag="ws")
                nc.sync.dma_start(out=ws[:, 0:HEADS * NMEM],
                                  in_=memk_ext[:, :, :].rearrange("p h c -> p (h c)"))
                nc.vector.tensor_copy(
                    out=kTp[:, :, 1024:1028],
                    in_=ws[:, 0:HEADS * NMEM].rearrange("p (h c) -> p h c", c=NMEM))
                for g in range(2):
                    ws = stage.tile([128, 1024], F32, tag="ws")
                    nc.sync.dma_start(out=ws[:, 0:VW], in_=memv_ext[:, g, :])
                    nc.vector.tensor_copy(out=vmem[:, g, :], in_=ws[:, 0:VW])
                for v in vexts:
                    oc = v[:, :, :].rearrange("p j (h c) -> p j h c", c=DH + 1)[:, :, :, DH:DH + 1]
                    nc.gpsimd.memset(oc, 1.0)

            # ---------------- pipeline stages ----------------
            def norm(bb, dmaq):
                """x -> xb (bf16 copy), xn (bf16 normalized)."""
                xb = xbp.tile([128, CT, N], BF16, tag="xb" + str(bb))
                for t in range(CT):
                    xs = stage.tile([128, 1024], F32, tag="xs")
                    dmaq.dma_start(out=xs, in_=x_ext[bb, t * 128:(t + 1) * 128, :])
                    nc.vector.tensor_copy(out=xb[:, t, :], in_=xs)
                xsq = data.tile([128, CT, N], FP8, tag="xsq")
                for t in range(CT):
                    nc.vector.tensor_mul(out=xsq[:, t, :], in0=xb[:, t, :], in1=xb[:, t, :])
                st = sim_ps.tile([128, 2, N], F32, tag="sim")
                ss = st[:, 0, :]
                for h2 in range(2):
                    for t in range(CT):
                        nc.tensor.matmul(ss[:, h2 * 512:(h2 + 1) * 512], ones128,
                                         xsq[:, t, h2 * 512:(h2 + 1) * 512],
                                         start=(t == 0), stop=(t == CT - 1))
                sroot = data.tile([128, N], F32, tag="sroot")
                nc.scalar.activation(out=sroot, in_=ss, func=AF.Sqrt, scale=1.0 / C)
                snorm = data.tile([128, N], F32, tag="snorm")
                nc.vector.reciprocal_approx_fast(out=snorm, in_=sroot)
                xn = data.tile([128, CT, N], BF16, tag="xn" + str(bb))
                eng = nc.vector if bb == 0 else nc.gpsimd
                for t in range(CT):
                    eng.tensor_mul(out=xn[:, t, :], in0=xb[:, t, :], in1=snorm)
                return xn

            def qkproj(xn, qT, mcs):
                """o-chunks mcs of the q/k projection; k goes into kTp (padded)."""
                for mc in mcs:
                    for h2 in range(2):
                        ps = qkv_ps.tile([128, 512], F32, tag="q")
                        for t in range(CT):
                            nc.tensor.matmul(ps, wqkv[:, t, mc * 128:(mc + 1) * 128],
                                             xn[:, t, h2 * 512:(h2 + 1) * 512],
                                             start=(t == 0), stop=(t == CT - 1))
                        if mc < 4:
                            nc.vector.tensor_copy(out=qT[:, mc, h2 * 512:(h2 + 1) * 512], in_=ps)
                        else:
                            h0, h1 = 2 * (mc - 4), 2 * (mc - 4) + 1
                            nc.vector.tensor_copy(
                                out=kTp[0:64, h0, h2 * 512:(h2 + 1) * 512], in_=ps[0:64, :])
                            nc.vector.tensor_copy(
                                out=kTp[64:128, h1, h2 * 512:(h2 + 1) * 512], in_=ps[64:128, :])

            def vproj(xn, vext, ics):
                for ic in ics:
                    ps = qkv_ps.tile([128, 512], F32, tag="q")
                    for t in range(CT):
                        nc.tensor.matmul(ps, xn[:, t, ic * 128:(ic + 1) * 128],
                                         wqkv[:, t, 2 * C:3 * C],
                                         start=(t == 0), stop=(t == CT - 1))
                    ps_h = ps[:, :].rearrange("p (h c) -> p h c", c=DH)
                    vdst = vext[:, ic, :].rearrange("p (h c) -> p h c", c=DH + 1)[:, :, 0:DH]
                    nc.vector.tensor_copy(out=vdst, in_=ps_h)

            def pair_sims(hp, qT, pjcs):
                """Row-tiled concurrent sims + merged exp for head pair hp."""
                he, ho = 2 * hp, 2 * hp + 1
                for jc in range(8):
                    st = sim_ps.tile([128, 2, N], F32, tag="sim")
                    with tc.high_priority():
                        for h2 in range(2):
                            nc.tensor.matmul(st[:, 0, h2 * 512:(h2 + 1) * 512],
                                             kTp[0:64, he, jc * 128:(jc + 1) * 128],
                                             qT[0:64, hp, h2 * 512:(h2 + 1) * 512],
                                             start=True, stop=True, tile_position=(0, 0))
                            nc.tensor.matmul(st[:, 1, h2 * 512:(h2 + 1) * 512],
                                             kTp[64:128, ho, jc * 128:(jc + 1) * 128],
                                             qT[64:128, hp, h2 * 512:(h2 + 1) * 512],
                                             start=True, stop=True, tile_position=(64, 0))
                    nc.scalar.activation(
                        out=pjcs[jc].rearrange("p a b -> p (a b)"),
                        in_=st.rearrange("p a b -> p (a b)"), func=AF.Exp)
                    hb = qkv_ps.tile([128, 512], F32, tag="q")
                    nc.tensor.matmul(hb[:, 0:128], ones128, ones128, start=True, stop=True)

            def pair_av(hp, pjcs, vext, attn, pmem):
                """av + normalization for head pair hp from buffered p chunks."""
                he, ho = 2 * hp, 2 * hp + 1
                for h2 in range(2):
                    avT = av_ps.tile([128, 2, 512], F32, tag="av")
                    for jc in range(8):
                        for par, h in ((0, he), (1, ho)):
                            nc.tensor.matmul(
                                avT[0:65, par, :],
                                vext[:, jc, h * (DH + 1):(h + 1) * (DH + 1)],
                                pjcs[jc][:, par, h2 * 512:(h2 + 1) * 512],
                                start=(jc == 0), stop=False)
                    for par, h in ((0, he), (1, ho)):
                        g, r0 = h // 4, 32 * (h % 4)
                        nc.tensor.matmul(
                            avT[0:65, par, :],
                            vmem[r0:r0 + NMEM, g, (h % 4) * (DH + 1):(h % 4 + 1) * (DH + 1)],
                            pmem[g][r0:r0 + NMEM, h2 * 512:(h2 + 1) * 512],
                            start=False, stop=True, tile_position=(r0, 0))
                    hb2 = qkv_ps.tile([128, 512], F32, tag="q")
                    nc.tensor.matmul(hb2[:, 0:128], ones128, ones128, start=True, stop=True)
                    avb = avsp.tile([128, 2, 512], F32R, tag="avs")
                    with tc.high_priority(offset=64):
                        nc.vector.tensor_copy(out=avb, in_=avT)
                    for par, h in ((0, he), (1, ho)):
                        bc = qkv_ps.tile([128, 512], F32, tag="q")
                        nc.tensor.matmul(bc[0:64, :], ones1[64:65, :],
                                         avb[64:65, par, :],
                                         start=True, stop=True)
                        rcp = rp.tile([64, 512], F32, tag="rcp")
                        nc.vector.reciprocal_approx_fast(out=rcp, in_=bc[0:64, :])
                        nc.vector.tensor_mul(
                            out=attn[64 * (h % 2):64 * (h % 2) + 64, hp,
                                     h2 * 512:(h2 + 1) * 512],
                            in0=avb[0:64, par, :].bitcast(F32), in1=rcp)

            def proj(attn, bb, mcs=None, h2s=(0, 1)):
                for mc in (range(CT) if mcs is None else mcs):
                    for h2 in h2s:
                        ps = qkv_ps.tile([128, 512], F32, tag="q")
                        for t in range(CT):
                            nc.tensor.matmul(ps, wo[:, t, mc * 128:(mc + 1) * 128],
                                             attn[:, t, h2 * 512:(h2 + 1) * 512],
                                             start=(t == 0), stop=(t == CT - 1))
                        ob = obp.tile([128, 512], F32, tag="ob")
                        nc.vector.tensor_copy(out=ob, in_=ps)
                        nc.sync.dma_start(
                            out=out_ext[bb, mc * 128:(mc + 1) * 128, h2 * 512:(h2 + 1) * 512],
                            in_=ob)

            def mem_sims(qT):
                pms = []
                for g in range(2):
                    st3 = sim_ps.tile([128, 2, N], F32, tag="sim")
                    st = st3[:, 0, :]
                    for h2 in range(2):
                        for h4 in range(4):
                            h = 4 * g + h4
                            nc.tensor.matmul(st[32 * h4:32 * h4 + NMEM, h2 * 512:(h2 + 1) * 512],
                                             kTp[:, h, 1024:1028],
                                             qT[:, h // 2, h2 * 512:(h2 + 1) * 512],
                                             start=True, stop=True, tile_position=(0, 32 * h4))
                    pmt = pm.tile([128, N], FP8, tag="pm")
                    nc.scalar.activation(out=pmt, in_=st, func=AF.Exp)
                    pms.append(pmt)
                return pms

            def alloc_pjcs():
                res = []
                for jc in range(8):
                    pt = pp.tile([128, 2, N], BF16, tag="pS", name=f"pS{jc}")
                    res.append(pt)
                return res

            # ---------------- schedule ----------------
            xn0 = norm(0, nc.sync)
            weight_prep()
            qT0 = qp.tile([128, CT, N], BF16, tag="qT")
            qT1 = qp.tile([128, CT, N], BF16, tag="qT")
            qkproj(xn0, qT0, range(0, 4))
            xn1 = norm(1, nc.scalar)
            qkproj(xn0, qT0, range(4, 8))
            vproj(xn0, vexts[0], range(8))

            pmem0 = mem_sims(qT0)
            attn0 = atp.tile([128, CT, N], BF16, tag="attn")
            attn1 = atp.tile([128, CT, N], BF16, tag="attn")
            pS_all = []
            for hp in range(4):
                if hp > 0:
                    pair_av(hp - 1, pS_all[hp - 1], vexts[0], attn0, pmem0)
                pS_all.append(alloc_pjcs())
                pair_sims(hp, qT0, pS_all[hp])
                # batch-1 projections fill the exp-bound bubbles; b1's k chunk
                # for this head pair is written after its sims finish reading.
                qkproj(xn1, qT1, [hp])
                vproj(xn1, vexts[1], [2 * hp, 2 * hp + 1])
                qkproj(xn1, qT1, [4 + hp])
            pmem1 = mem_sims(qT1)
            pair_av(3, pS_all[3], vexts[0], attn0, pmem0)

            for hp in range(4):
                if hp > 0:
                    pair_av(hp - 1, pS_all[4 + hp - 1], vexts[1], attn1, pmem1)
                pS_all.append(alloc_pjcs())
                pair_sims(hp, qT1, pS_all[4 + hp])
                proj(attn0, 0, [hp])
            pair_av(3, pS_all[7], vexts[1], attn1, pmem1)
            proj(attn1, 1)
    nc.compile()
    return nc


_NC_CACHE = []


def kernel(x, gamma, mem_kv, w_qkv, w_out, _trace=False):
    x = np.asarray(x, dtype=np.float32)
    gamma = np.asarray(gamma, dtype=np.float32)
    mem_kv = np.asarray(mem_kv, dtype=np.float32)
    w_qkv = np.asarray(w_qkv, dtype=np.float32)
    w_out = np.asarray(w_out, dtype=np.float32)

    b, c, hh, ww = x.shape
    n = hh * ww
    xs = x.reshape(b, c, n)

    wqkvt = np.ascontiguousarray(w_qkv.T)          # [c, 3c]
    wot = np.ascontiguousarray(w_out.T)            # [c, c]
    gammat = np.ascontiguousarray(gamma.reshape(CT, 128).T)  # [128, CT]

    memk = np.zeros((128, HEADS, NMEM), np.float32)
    memv = np.zeros((128, 2, VW), np.float32)
    for h in range(HEADS):
        r0 = 64 * (h % 2)
        memk[r0:r0 + DH, h, 0:NMEM] = mem_kv[0, h].T      # [dh, nmem]
        g, r1, c0 = h // 4, 32 * (h % 4), (h % 4) * (DH + 1)
        memv[r1:r1 + NMEM, g, c0:c0 + DH] = mem_kv[1, h]
        memv[r1:r1 + NMEM, g, c0 + DH] = 1.0

    if not _NC_CACHE:
        _NC_CACHE.append(_build())
    nc = _NC_CACHE[0]

    in_maps = []
    for core in range(NCORES):
        in_maps.append({
            "x": np.ascontiguousarray(xs[core * PB:(core + 1) * PB]),
            "wqkvt": wqkvt,
            "wot": wot,
            "gammat": gammat,
            "memk": memk,
            "memv": memv,
        })
    res = run_bass_kernel_spmd(nc, in_maps, core_ids=list(range(NCORES)), trace=_trace)
    out = np.concatenate([res.results[core]["out"] for core in range(NCORES)], axis=0)
    kernel.last_result = res
    return out.reshape(b, c, hh, ww)


# revision 10
# speedup vs baseline: 1.2122x; 1.2122x over previous
"""Trainium2 Bass kernel for nn_Attention_7945689497706.

Distribution: data-parallel over batch, 2 batch elements per core, weights
replicated, no collectives.

Per-core layout (v3):
  - PE warm-up matmuls during the initial x DMA (HAM un-throttle).
  - x staged in f32 chunks then cast to bf16 once; norm squares/muls run off
    the bf16 copy (xn mul on GPSIMD to unload the DVE).
  - Both batches' sqrt-based norms run before any exp, so the ACT table set
    switches twice total instead of per batch.
  - sims for a head pair run as two concurrent 64-row PE tiles into one
    [128, 2, 1024] psum tile (high_priority keeps the quad adjacent in the
    engine queue); one [128, 2048] exp per (pair, jc) evacuates both heads.
  - p chunks live in a ring of per-jc [128, 2, 1024] bf16 tiles; av replays
    them after the pair's exps (h2-major phases, one [128, 2, 512] av psum).
  - denominator rides the 65th stationary column; bc broadcast + fast
    reciprocal + mul as in the baseline.
  - batch-1 projections interleave into batch-0's attention (and batch-0
    out-proj into batch-1's) to fill the ACT-bound exp bubbles.
"""

import numpy as np

import concourse.bass as bass
import concourse.mybir as mybir
import concourse.tile as tile
from concourse import bacc
from concourse.bass_utils import run_bass_kernel_spmd

F32 = mybir.dt.float32
F32R = mybir.dt.float32r
BF16 = mybir.dt.bfloat16
FP8 = mybir.dt.float8e4
AF = mybir.ActivationFunctionType

NCORES = 8
B = 16
C = 512
N = 1024          # pixels = 32*32
HEADS = 8
DH = 64
NMEM = 4
PB = B // NCORES  # batch elements per core
CT = C // 128     # channel partition-tiles
VW = HEADS * (DH + 1)  # per head [v | ones] = 65


def _build():
    nc = bacc.Bacc()
    x_ext = nc.declare_dram_parameter("x", [PB, C, N], F32, isOutput=False)
    wqkvt_ext = nc.declare_dram_parameter("wqkvt", [C, 3 * C], F32, isOutput=False)
    wot_ext = nc.declare_dram_parameter("wot", [C, C], F32, isOutput=False)
    gammat_ext = nc.declare_dram_parameter("gammat", [128, CT], F32, isOutput=False)
    memk_ext = nc.declare_dram_parameter("memk", [128, HEADS, NMEM], F32, isOutput=False)
    memv_ext = nc.declare_dram_parameter("memv", [128, 2, VW], F32, isOutput=False)
    out_ext = nc.declare_dram_parameter("out", [PB, C, N], F32, isOutput=True)

    with tile.TileContext(nc) as tc:
        with (
            tc.tile_pool(name="const", bufs=1) as const,
            tc.tile_pool(name="stage", bufs=2) as stage,
            tc.tile_pool(name="xbp", bufs=1) as xbp,
            tc.tile_pool(name="data", bufs=1) as data,
            tc.tile_pool(name="atp", bufs=2) as atp,
            tc.tile_pool(name="qp", bufs=2) as qp,
            tc.tile_pool(name="pp", bufs=12) as pp,
            tc.tile_pool(name="pm", bufs=4) as pm,
            tc.tile_pool(name="avs", bufs=2) as avsp,
            tc.tile_pool(name="rp", bufs=1) as rp,
            tc.tile_pool(name="ob", bufs=1) as obp,
            tc.tile_pool(name="qkv_ps", bufs=2, space="PSUM") as qkv_ps,
            tc.tile_pool(name="sim_ps", bufs=1, space="PSUM") as sim_ps,
            tc.tile_pool(name="av_ps", bufs=1, space="PSUM") as av_ps,
        ):
            # ---------------- constants + PE warm-up ----------------
            ones128 = const.tile([128, 128], BF16, tag="ones128")
            nc.vector.memset(ones128, 1.0)
            for w in range(60):
                wp = qkv_ps.tile([128, 512], F32, tag="q")
                nc.tensor.matmul(wp[:, 0:128], ones128, ones128, start=True, stop=True)

            wqkv = const.tile([128, CT, 3 * C], BF16, tag="wqkv")
            wo = const.tile([128, CT, C], BF16, tag="wo")
            g1 = const.tile([128, CT], F32, tag="g1")
            g1q = const.tile([128, CT], F32, tag="g1q")
            ones1 = const.tile([128, 64], F32R, tag="ones1")
            kTp = const.tile([128, HEADS, 1028], BF16, tag="kTp")
            vextA = const.tile([128, 8, VW], BF16, tag="vextA")
            vextB = const.tile([128, 8, VW], BF16, tag="vextB")
            vmem = const.tile([128, 2, VW], FP8, tag="vmem")
            vexts = [vextA, vextB]

            gsb = const.tile([128, CT], F32, tag="gsb")
            nc.sync.dma_start(out=gsb, in_=gammat_ext[:, :])
            nc.scalar.activation(out=g1, in_=gsb, func=AF.Copy, bias=1.0)
            nc.scalar.activation(out=g1q, in_=gsb, func=AF.Copy, bias=1.0)
            nc.scalar.mul(out=g1q, in_=g1q, mul=DH ** -0.5)
            nc.vector.memset(ones1.bitcast(F32), 1.0)

            def weight_prep():
                nc.gpsimd.memset(kTp, 0.0)
                for t in range(CT):
                    for half in range(2):
                        ws = stage.tile([128, 1024], F32, tag="ws")
                        c0 = half * 768
                        nc.sync.dma_start(out=ws[:, 0:768],
                                          in_=wqkvt_ext[t * 128:(t + 1) * 128, c0:c0 + 768])
                        if half == 0:
                            nc.vector.tensor_scalar_mul(
                                out=wqkv[:, t, 0:C], in0=ws[:, 0:C], scalar1=g1q[:, t:t + 1])
                            nc.vector.tensor_scalar_mul(
                                out=wqkv[:, t, C:768], in0=ws[:, C:768], scalar1=g1[:, t:t + 1])
                        else:
                            nc.vector.tensor_scalar_mul(
                                out=wqkv[:, t, 768:3 * C], in0=ws[:, 0:768],
                                scalar1=g1[:, t:t + 1])
                for t in range(CT):
                    ws = stage.tile([128, 1024], F32, tag="ws")
                    nc.sync.dma_start(out=ws[:, 0:C], in_=wot_ext[t * 128:(t + 1) * 128, :])
                    nc.vector.tensor_copy(out=wo[:, t, :], in_=ws[:, 0:C])
                ws = stage.tile([128, 1024], F32, tag="ws")
                nc.sync.dma_start(out=ws[:, 0:HEADS * NMEM],
                                  in_=memk_ext[:, :, :].rearrange("p h c -> p (h c)"))
                nc.vector.tensor_copy(
                    out=kTp[:, :, 1024:1028],
                    in_=ws[:, 0:HEADS * NMEM].rearrange("p (h c) -> p h c", c=NMEM))
                for g in range(2):
                    ws = stage.tile([128, 1024], F32, tag="ws")
                    nc.sync.dma_start(out=ws[:, 0:VW], in_=memv_ext[:, g, :])
                    nc.vector.tensor_copy(out=vmem[:, g, :], in_=ws[:, 0:VW])
                for v in vexts:
                    oc = v[:, :, :].rearrange("p j (h c) -> p j h c", c=DH + 1)[:, :, :, DH:DH + 1]
                    nc.gpsimd.memset(oc, 1.0)

            # ---------------- pipeline stages ----------------
            def norm(bb, dmaq):
                """x -> xb (bf16 copy), xn (bf16 normalized)."""
                xb = xbp.tile([128, CT, N], BF16, tag="xb" + str(bb))
                for t in range(CT):
                    xs = stage.tile([128, 1024], F32, tag="xs")
                    dmaq.dma_start(out=xs, in_=x_ext[bb, t * 128:(t + 1) * 128, :])
                    nc.vector.tensor_copy(out=xb[:, t, :], in_=xs)
                xsq = data.tile([128, CT, N], FP8, tag="xsq")
                for t in range(CT):
                    nc.vector.tensor_mul(out=xsq[:, t, :], in0=xb[:, t, :], in1=xb[:, t, :])
                st = sim_ps.tile([128, 2, N], F32, tag="sim")
                ss = st[:, 0, :]
                for h2 in range(2):
                    for t in range(CT):
                        nc.tensor.matmul(ss[:, h2 * 512:(h2 + 1) * 512], ones128,
                                         xsq[:, t, h2 * 512:(h2 + 1) * 512],
                                         start=(t == 0), stop=(t == CT - 1))
                sroot = data.tile([128, N], F32, tag="sroot")
                nc.scalar.activation(out=sroot, in_=ss, func=AF.Sqrt, scale=1.0 / C)
                snorm = data.tile([128, N], F32, tag="snorm")
                nc.vector.reciprocal_approx_fast(out=snorm, in_=sroot)
                xn = data.tile([128, CT, N], BF16, tag="xn" + str(bb))
                eng = nc.vector if bb == 0 else nc.gpsimd
                for t in range(CT):
                    eng.tensor_mul(out=xn[:, t, :], in0=xb[:, t, :], in1=snorm)
                return xn

            def qkproj(xn, qT, mcs):
                """o-chunks mcs of the q/k projection; k goes into kTp (padded)."""
                for mc in mcs:
                    for h2 in range(2):
                        ps = qkv_ps.tile([128, 512], F32, tag="q")
                        for t in range(CT):
                            nc.tensor.matmul(ps, wqkv[:, t, mc * 128:(mc + 1) * 128],
                                             xn[:, t, h2 * 512:(h2 + 1) * 512],
                                             start=(t == 0), stop=(t == CT - 1))
                        if mc < 4:
                            nc.vector.tensor_copy(out=qT[:, mc, h2 * 512:(h2 + 1) * 512], in_=ps)
                        else:
                            h0, h1 = 2 * (mc - 4), 2 * (mc - 4) + 1
                            nc.vector.tensor_copy(
                                out=kTp[0:64, h0, h2 * 512:(h2 + 1) * 512], in_=ps[0:64, :])
                            nc.vector.tensor_copy(
                                out=kTp[64:128, h1, h2 * 512:(h2 + 1) * 512], in_=ps[64:128, :])

            def vproj(xn, vext, ics):
                for ic in ics:
                    ps = qkv_ps.tile([128, 512], F32, tag="q")
                    for t in range(CT):
                        nc.tensor.matmul(ps, xn[:, t, ic * 128:(ic + 1) * 128],
                                         wqkv[:, t, 2 * C:3 * C],
                                         start=(t == 0), stop=(t == CT - 1))
                    ps_h = ps[:, :].rearrange("p (h c) -> p h c", c=DH)
                    vdst = vext[:, ic, :].rearrange("p (h c) -> p h c", c=DH + 1)[:, :, 0:DH]
                    nc.vector.tensor_copy(out=vdst, in_=ps_h)

            def pair_sims(hp, qT, pjcs):
                """Row-tiled concurrent sims + merged exp for head pair hp."""
                he, ho = 2 * hp, 2 * hp + 1
                for jc in range(8):
                    st = sim_ps.tile([128, 2, N], F32, tag="sim")
                    with tc.high_priority():
                        for h2 in range(2):
                            nc.tensor.matmul(st[:, 0, h2 * 512:(h2 + 1) * 512],
                                             kTp[0:64, he, jc * 128:(jc + 1) * 128],
                                             qT[0:64, hp, h2 * 512:(h2 + 1) * 512],
                                             start=True, stop=True, tile_position=(0, 0))
                            nc.tensor.matmul(st[:, 1, h2 * 512:(h2 + 1) * 512],
                                             kTp[64:128, ho, jc * 128:(jc + 1) * 128],
                                             qT[64:128, hp, h2 * 512:(h2 + 1) * 512],
                                             start=True, stop=True, tile_position=(64, 0))
                    nc.scalar.activation(
                        out=pjcs[jc].rearrange("p a b -> p (a b)"),
                        in_=st.rearrange("p a b -> p (a b)"), func=AF.Exp)
                    hb = qkv_ps.tile([128, 512], F32, tag="q")
                    nc.tensor.matmul(hb[:, 0:128], ones128, ones128, start=True, stop=True)

            def pair_av(hp, pjcs, vext, attn, pmem):
                """av + normalization for head pair hp from buffered p chunks."""
                he, ho = 2 * hp, 2 * hp + 1
                for h2 in range(2):
                    avT = av_ps.tile([128, 2, 512], F32, tag="av")
                    for jc in range(8):
                        for par, h in ((0, he), (1, ho)):
                            nc.tensor.matmul(
                                avT[0:65, par, :],
                                vext[:, jc, h * (DH + 1):(h + 1) * (DH + 1)],
                                pjcs[jc][:, par, h2 * 512:(h2 + 1) * 512],
                                start=(jc == 0), stop=False)
                    for par, h in ((0, he), (1, ho)):
                        g, r0 = h // 4, 32 * (h % 4)
                        nc.tensor.matmul(
                            avT[0:65, par, :],
                            vmem[r0:r0 + NMEM, g, (h % 4) * (DH + 1):(h % 4 + 1) * (DH + 1)],
                            pmem[g][r0:r0 + NMEM, h2 * 512:(h2 + 1) * 512],
                            start=False, stop=True, tile_position=(r0, 0))
                    hb2 = qkv_ps.tile([128, 512], F32, tag="q")
                    nc.tensor.matmul(hb2[:, 0:128], ones128, ones128, start=True, stop=True)
                    avb = avsp.tile([128, 2, 512], F32R, tag="avs")
                    with tc.high_priority(offset=64):
                        nc.vector.tensor_copy(out=avb, in_=avT)
                    for par, h in ((0, he), (1, ho)):
                        bc = qkv_ps.tile([128, 512], F32, tag="q")
                        nc.tensor.matmul(bc[0:64, :], ones1[64:65, :],
                                         avb[64:65, par, :],
                                         start=True, stop=True)
                        rcp = rp.tile([64, 512], F32, tag="rcp")
                        nc.vector.reciprocal_approx_fast(out=rcp, in_=bc[0:64, :])
                        nc.vector.tensor_mul(
                            out=attn[64 * (h % 2):64 * (h % 2) + 64, hp,
                                     h2 * 512:(h2 + 1) * 512],
                            in0=avb[0:64, par, :].bitcast(F32), in1=rcp)

            def proj(attn, bb, mcs=None, h2s=(0, 1)):
                for mc in (range(CT) if mcs is None else mcs):
                    for h2 in h2s:
                        ps = qkv_ps.tile([128, 512], F32, tag="q")
                        for t in range(CT):
                            nc.tensor.matmul(ps, wo[:, t, mc * 128:(mc + 1) * 128],
                                             attn[:, t, h2 * 512:(h2 + 1) * 512],
                                             start=(t == 0), stop=(t == CT - 1))
                        ob = obp.tile([128, 512], F32, tag="ob")
                        nc.vector.tensor_copy(out=ob, in_=ps)
                        nc.sync.dma_start(
                            out=out_ext[bb, mc * 128:(mc + 1) * 128, h2 * 512:(h2 + 1) * 512],
                            in_=ob)

            def mem_sims(qT):
                pms = []
                for g in range(2):
                    st3 = sim_ps.tile([128, 2, N], F32, tag="sim")
                    st = st3[:, 0, :]
                    for h2 in range(2):
                        for h4 in range(4):
                            h = 4 * g + h4
                            nc.tensor.matmul(st[32 * h4:32 * h4 + NMEM, h2 * 512:(h2 + 1) * 512],
                                             kTp[:, h, 1024:1028],
                                             qT[:, h // 2, h2 * 512:(h2 + 1) * 512],
                                             start=True, stop=True, tile_position=(0, 32 * h4))
                    pmt = pm.tile([128, N], FP8, tag="pm")
                    nc.scalar.activation(out=pmt, in_=st, func=AF.Exp)
                    pms.append(pmt)
                return pms

            def alloc_pjcs():
                res = []
                for jc in range(8):
                    pt = pp.tile([128, 2, N], BF16, tag="pS", name=f"pS{jc}")
                    res.append(pt)
                return res

            # ---------------- schedule ----------------
            xn0 = norm(0, nc.sync)
            weight_prep()
            qT0 = qp.tile([128, CT, N], BF16, tag="qT")
            qT1 = qp.tile([128, CT, N], BF16, tag="qT")
            qkproj(xn0, qT0, range(0, 4))
            xn1 = norm(1, nc.scalar)
            qkproj(xn0, qT0, range(4, 8))
            vproj(xn0, vexts[0], range(8))

            pmem0 = mem_sims(qT0)
            attn0 = atp.tile([128, CT, N], BF16, tag="attn")
            attn1 = atp.tile([128, CT, N], BF16, tag="attn")
            pS_all = []
            for hp in range(4):
                if hp > 0:
                    pair_av(hp - 1, pS_all[hp - 1], vexts[0], attn0, pmem0)
                pS_all.append(alloc_pjcs())
                pair_sims(hp, qT0, pS_all[hp])
                # batch-1 projections fill the exp-bound bubbles; b1's k chunk
                # for this head pair is written after its sims finish reading.
                qkproj(xn1, qT1, [hp])
                vproj(xn1, vexts[1], [2 * hp, 2 * hp + 1])
                qkproj(xn1, qT1, [4 + hp])
            pmem1 = mem_sims(qT1)
            pair_av(3, pS_all[3], vexts[0], attn0, pmem0)

            for hp in range(4):
                if hp > 0:
                    pair_av(hp - 1, pS_all[4 + hp - 1], vexts[1], attn1, pmem1)
                pS_all.append(alloc_pjcs())
                pair_sims(hp, qT1, pS_all[4 + hp])
                proj(attn0, 0, [hp])
            pair_av(3, pS_all[7], vexts[1], attn1, pmem1)
            proj(attn1, 1)
    nc.compile()
    return nc


_NC_CACHE = []


def kernel(x, gamma, mem_kv, w_qkv, w_out, _trace=False):
    x = np.asarray(x, dtype=np.float32)
    gamma = np.asarray(gamma, dtype=np.float32)
    mem_kv = np.asarray(mem_kv, dtype=np.float32)
    w_qkv = np.asarray(w_qkv, dtype=np.float32)
    w_out = np.asarray(w_out, dtype=np.float32)

    b, c, hh, ww = x.shape
    n = hh * ww
    xs = x.reshape(b, c, n)

    wqkvt = np.ascontiguousarray(w_qkv.T)          # [c, 3c]
    wot = np.ascontiguousarray(w_out.T)            # [c, c]
    gammat = np.ascontiguousarray(gamma.reshape(CT, 128).T)  # [128, CT]

    memk = np.zeros((128, HEADS, NMEM), np.float32)
    memv = np.zeros((128, 2, VW), np.float32)
    for h in range(HEADS):
        r0 = 64 * (h % 2)
        memk[r0:r0 + DH, h, 0:NMEM] = mem_kv[0, h].T      # [dh, nmem]
        g, r1, c0 = h // 4, 32 * (h % 4), (h % 4) * (DH + 1)
        memv[r1:r1 + NMEM, g, c0:c0 + DH] = mem_kv[1, h]
        memv[r1:r1 + NMEM, g, c0 + DH] = 1.0

    if not _NC_CACHE:
        _NC_CACHE.append(_build())
    nc = _NC_CACHE[0]

    in_maps = []
    for core in range(NCORES):
        in_maps.append({
            "x": np.ascontiguousarray(xs[core * PB:(core + 1) * PB]),
            "wqkvt": wqkvt,
            "wot": wot,
            "gammat": gammat,
            "memk": memk,
            "memv": memv,
        })
    res = run_bass_kernel_spmd(nc, in_maps, core_ids=list(range(NCORES)), trace=_trace)
    out = np.concatenate([res.results[core]["out"] for core in range(NCORES)], axis=0)
    kernel.last_result = res
    return out.reshape(b, c, hh, ww)
